# revision 5
# baseline (speedup 1.0000x reference)
"""Trainium2 Bass kernel for the Cheirality loss layer (v7).

Math (per batch b, pixel (y, x); g = grad_dirs, n = normal_flow):
    AV0 = V2*x - V0                    AV1 = V2*y - V1
    BW0 = O0*x*y - O1*(x^2+1) + O2*y   BW1 = O0*(y^2+1) - O1*x*y - O2*x
    rho = (g0*AV0 + g1*AV1) * (n0 + n1 - g0*BW0 - g1*BW1)
    out = mean(gelu(-rho))             (exact erf-based gelu)

v7 changes vs v6 (45.3us baseline):
  - column-major pixel layout: partition p = 64*h + x//10, f = y*10 + x%10.
    x = 10*(p%64) + j is affine per partition, y = f//10 identical on all
    partitions -> the x/y grids no longer ship from HBM (2.46MB saved):
    a [1,2,4800] fp16 row pair (j, y) is DMA-broadcast (stride-0 partition
    src) to all 128 partitions and xg = jg + 10*(p%64) via one 4x
    tensor_scalar.
  - normal_flow ships as fp8e4m3 (1.23MB instead of 2.46MB); it is consumed
    only by the PE as a single DoubleRow matmul computing -(n0+n1) in one
    FC-column pass (fp8 pair-contraction), removing 2 bf16 terms.
  - gd + nf packed into ONE dram stream per chunk (uint8, bitcast views) ->
    1 DMA descriptor per chunk instead of 3 (sync-engine desc-gen was 15us).
  - PE term order groups same-diag matmuls for LDWEIGHTS reuse.
Device dataflow per chunk:
    DVE: P1=g0*xg P2=g1*yg u=P1+P2 XU=xg*u YU=yg*u P3=g0*yg P4=g1*xg (bf16 2x)
    PE:  d1m  = V0*g0 + V1*g1 - V2*u            ( = -dot1 )
         NEG  = -(n0+n1) [DoubleRow fp8] + O0*g1 - O1*g0 + O2*P3 - O2*P4
                + O0*YU - O1*XU               ( = g.BW - n0 - n1 = -r2 )
    ACT: negb/d1b PSUM->bf16 copies; DVE rho = d1b*negb; ACT gelu(-rho)
    with accum_out -> [128, NCHUNK] partials, host sums in float64.
Sharding: pure data parallel, 2 batches per core on partition halves.
"""

import numpy as np
import ml_dtypes

import concourse.bacc as bacc
import concourse.bass as bass
import concourse.tile as tile
from concourse import mybir
from concourse.bass_utils import run_bass_kernel_spmd

B, H, W = 16, 480, 640
NPIX = H * W            # 307200
NCORES = 8
BPC = B // NCORES       # 2 batches per core
PHALF = 64              # partitions per batch
CPP = W // PHALF        # 10 image columns per partition
FTOT = H * CPP          # 4800 free elems per partition
CHUNKS = [480, 960, 960, 960, 960, 480]
NCHUNK = len(CHUNKS)
FCMAX = max(CHUNKS)
MMF = 512               # max matmul free dim (one PSUM bank)

F32 = mybir.dt.float32
F16 = mybir.dt.float16
BF16 = mybir.dt.bfloat16
F8 = mybir.dt.float8e4
U8 = mybir.dt.uint8
AF = mybir.ActivationFunctionType

# fp16 diag slots
D_O0, D_O1N, D_O2, D_O2N, D_V0, D_V1, D_V2N = range(7)
NDIAG = 7


def _build_kernel(tc, gn, grids, diags, wdr, xp, out):
    nc = tc.nc
    gn_t = gn.ap()

    with (
        tc.tile_pool(name="singles", bufs=1) as singles,
        tc.tile_pool(name="ins", bufs=3) as ins,
        tc.tile_pool(name="mids", bufs=2) as mids,
        tc.tile_pool(name="psum", bufs=2, space="PSUM") as psp,
    ):
        grid_t = singles.tile([128, 2, FTOT], F16, name="grid_t")
        xg = singles.tile([128, FTOT], F16, name="xg")
        dg = singles.tile([128, NDIAG, 128], F16, name="dg")
        wdrt = singles.tile([128, 2, 128], F8, name="wdrt")
        xpt = singles.tile([128, 1], F32, name="xpt")
        acc = singles.tile([128, NCHUNK], F32, name="acc")

        # one-shot loads on the scalar ring (sync ring carries the bulk data)
        nc.scalar.dma_start(out=grid_t, in_=grids.ap().partition_broadcast(128))
        nc.scalar.dma_start(out=dg, in_=diags.ap().rearrange("d k m -> k d m"))
        nc.scalar.dma_start(out=wdrt, in_=wdr.ap())
        nc.scalar.dma_start(out=xpt, in_=xp.ap())
        DG = [dg[:, i, :] for i in range(NDIAG)]

        # xg = jg + 10*(p%64)  (one 4x tensor_scalar; exact in fp16)
        nc.vector.tensor_scalar_add(xg, grid_t[:, 0], xpt)
        yg = grid_t[:, 1]

        f0s = [sum(CHUNKS[:i]) for i in range(NCHUNK)]
        for ci in range(NCHUNK):
            FC = CHUNKS[ci]
            f0 = f0s[ci]
            sl = slice(f0, f0 + FC)
            gnt = ins.tile([128, 6 * FCMAX], U8, tag="gn", name=f"gn_{ci}")
            nc.sync.dma_start(
                out=gnt[:, : 6 * FC], in_=gn_t[:, 6 * f0 : 6 * f0 + 6 * FC]
            )
            g0 = gnt[:, 0 : 2 * FC].bitcast(BF16)
            g1 = gnt[:, 2 * FC : 4 * FC].bitcast(BF16)
            npair = gnt[:, 4 * FC : 6 * FC].bitcast(F8).rearrange(
                "p (f k) -> p k f", k=2
            )
            xc = xg[:, sl]
            yc = yg[:, sl]

            def mtile(tag, dt=BF16):
                return mids.tile([128, FCMAX], dt, tag=tag, name=f"{tag}_{ci}")[:, :FC]

            P1 = mtile("P1")
            nc.vector.tensor_mul(out=P1, in0=g0, in1=xc)
            P2 = mtile("P2")
            nc.vector.tensor_mul(out=P2, in0=g1, in1=yc)
            u = mtile("u")
            nc.vector.tensor_add(out=u, in0=P1, in1=P2)
            P3 = mtile("P3")
            nc.vector.tensor_mul(out=P3, in0=g0, in1=yc)
            P4 = mtile("P4")
            nc.vector.tensor_mul(out=P4, in0=g1, in1=xc)
            XU = mtile("XU")
            nc.vector.tensor_mul(out=XU, in0=xc, in1=u)
            YU = mtile("YU")
            nc.vector.tensor_mul(out=YU, in0=yc, in1=u)

            # PE: NEG = -(n0+n1) + g.BW ; d1m = V0*g0 + V1*g1 - V2*u
            neg_ps = psp.tile([128, FCMAX], F32, tag="neg", name=f"neg_{ci}")[:, :FC]
            d1_ps = psp.tile([128, FCMAX], F32, tag="d1", name=f"d1_{ci}")[:, :FC]

            # DoubleRow: out[m,f] = sum_k wdr[c,(k,m)]*npair[c,k,f] = -(n0+n1)
            for f0m in range(0, FC, MMF):
                fs = slice(f0m, min(f0m + MMF, FC))
                nc.tensor.matmul(
                    neg_ps[:, fs], wdrt[:, :, :], npair[:, :, fs],
                    start=True, stop=False,
                    perf_mode=mybir.MatmulPerfMode.DoubleRow,
                )
            neg_terms = [
                (D_O0, g1), (D_O1N, g0),
                (D_O2, P3), (D_O2N, P4),
                (D_O0, YU), (D_O1N, XU),
            ]
            for i, (di, rhs) in enumerate(neg_terms):
                for f0m in range(0, FC, MMF):
                    fs = slice(f0m, min(f0m + MMF, FC))
                    nc.tensor.matmul(
                        neg_ps[:, fs], DG[di], rhs[:, fs],
                        start=False, stop=(i == len(neg_terms) - 1),
                    )
            d1_terms = [(D_V0, g0), (D_V1, g1), (D_V2N, u)]
            for i, (di, rhs) in enumerate(d1_terms):
                for f0m in range(0, FC, MMF):
                    fs = slice(f0m, min(f0m + MMF, FC))
                    nc.tensor.matmul(
                        d1_ps[:, fs], DG[di], rhs[:, fs],
                        start=(i == 0), stop=(i == len(d1_terms) - 1),
                    )

            negb = mtile("negb")
            nc.scalar.activation(out=negb, in_=neg_ps, func=AF.Copy)
            d1b = mtile("d1b")
            nc.scalar.activation(out=d1b, in_=d1_ps, func=AF.Copy)

            rho = mtile("rho")
            nc.vector.tensor_mul(out=rho, in0=d1b, in1=negb)
            gl = mtile("gl")
            nc.scalar.activation(
                out=gl, in_=rho, func=AF.Gelu, bias=0.0, scale=-1.0,
                accum_out=acc[:, ci : ci + 1],
            )

        nc.sync.dma_start(out=out.ap(), in_=acc)


def build_bass():
    nc = bacc.Bacc("TRN2", target_bir_lowering=False, debug=False)
    gn = nc.dram_tensor("gn", [128, 6 * FTOT], U8, kind="ExternalInput")
    grids = nc.dram_tensor("grids", [1, 2, FTOT], F16, kind="ExternalInput")
    diags = nc.dram_tensor("diags", [NDIAG, 128, 128], F16, kind="ExternalInput")
    wdr = nc.dram_tensor("wdr", [128, 2, 128], F8, kind="ExternalInput")
    xp = nc.dram_tensor("xp", [128, 1], F32, kind="ExternalInput")
    out = nc.dram_tensor("acc_out", [128, NCHUNK], F32, kind="ExternalOutput")
    with tile.TileContext(nc) as tc:
        _build_kernel(tc, gn, grids, diags, wdr, xp, out)
    nc.compile()
    return nc


def make_in_maps(pose, grad_dirs, normal_flow):
    pose = np.asarray(pose, np.float32)
    gd = np.ascontiguousarray(np.asarray(grad_dirs, np.float32))
    nf = np.ascontiguousarray(np.asarray(normal_flow, np.float32))

    f = np.arange(FTOT, dtype=np.int64)
    grids = np.stack([(f % CPP), (f // CPP)], axis=0).astype(np.float16)
    grids = np.ascontiguousarray(grids.reshape(1, 2, FTOT))
    xp = ((np.arange(128) % PHALF) * CPP).astype(np.float32).reshape(128, 1)

    f0s = [sum(CHUNKS[:i]) for i in range(NCHUNK)]

    def col_interleave(a, dtype):
        # [BPC, 2, H, W] -> [128, 2, FTOT]; partition = 64*h + x//10,
        # f = y*10 + x%10
        return np.ascontiguousarray(
            a.reshape(BPC, 2, H, PHALF, CPP)
            .transpose(0, 3, 1, 2, 4)
            .reshape(128, 2, FTOT)
            .astype(dtype)
        )

    in_maps = []
    for core in range(NCORES):
        b0 = core * BPC
        gdc = col_interleave(gd[b0 : b0 + BPC], ml_dtypes.bfloat16)
        nfc = col_interleave(nf[b0 : b0 + BPC], ml_dtypes.float8_e4m3)
        # packed per-chunk stream: [g0 bf16 | g1 bf16 | (n0,n1) fp8 pairs]
        gn = np.empty((128, 6 * FTOT), np.uint8)
        for ci in range(NCHUNK):
            FC = CHUNKS[ci]
            f0 = f0s[ci]
            b = 6 * f0
            sl = slice(f0, f0 + FC)
            gn[:, b : b + 2 * FC] = gdc[:, 0, sl].view(np.uint8)
            gn[:, b + 2 * FC : b + 4 * FC] = gdc[:, 1, sl].view(np.uint8)
            gn[:, b + 4 * FC : b + 6 * FC] = np.ascontiguousarray(
                nfc[:, :, sl].transpose(0, 2, 1)
            ).reshape(128, 2 * FC).view(np.uint8)

        coef = np.zeros((NDIAG, 128), np.float32)
        for h in range(BPC):
            V = pose[b0 + h, :3]
            O = pose[b0 + h, 3:]
            rows = slice(h * PHALF, (h + 1) * PHALF)
            coef[D_O0, rows] = O[0]
            coef[D_O1N, rows] = -O[1]
            coef[D_O2, rows] = O[2]
            coef[D_O2N, rows] = -O[2]
            coef[D_V0, rows] = V[0]
            coef[D_V1, rows] = V[1]
            coef[D_V2N, rows] = -V[2]
        diags = np.zeros((NDIAG, 128, 128), np.float16)
        for i in range(NDIAG):
            np.fill_diagonal(diags[i], coef[i].astype(np.float16))
        # DoubleRow weights: wdr[c, k*128+m] = -1 if m == c (both k)
        wdr = np.zeros((128, 2, 128), ml_dtypes.float8_e4m3)
        for c in range(128):
            wdr[c, 0, c] = -1.0
            wdr[c, 1, c] = -1.0
        in_maps.append(
            {
                "gn": gn,
                "grids": grids,
                "diags": diags,
                "wdr": wdr,
                "xp": xp,
            }
        )
    return in_maps


_NC_CACHE = None


def _get_nc():
    global _NC_CACHE
    if _NC_CACHE is None:
        _NC_CACHE = build_bass()
    return _NC_CACHE


def kernel(pose, grad_dirs, normal_flow):
    nc = _get_nc()
    in_maps = make_in_maps(pose, grad_dirs, normal_flow)
    res = run_bass_kernel_spmd(nc, in_maps, core_ids=list(range(NCORES)))
    total = 0.0
    for r in res.results:
        total += r["acc_out"].astype(np.float64).sum()
    return np.float32(total / (B * H * W))


# revision 6
# speedup vs baseline: 1.3239x; 1.3239x over previous
"""Trainium2 Bass kernel for the Cheirality loss layer (v7.1).

Math (per batch b, pixel (y, x); g = grad_dirs, n = normal_flow):
    AV0 = V2*x - V0                    AV1 = V2*y - V1
    BW0 = O0*x*y - O1*(x^2+1) + O2*y   BW1 = O0*(y^2+1) - O1*x*y - O2*x
    rho = (g0*AV0 + g1*AV1) * (n0 + n1 - g0*BW0 - g1*BW1)
    out = mean(gelu(-rho))             (exact erf-based gelu)

Layout: column-major pixels: partition p = 64*h + x//10, f = y*10 + x%10.
  -> x = 10*(p%64) + (f%10): periodic-10, served by a [128,10] fp16 tile
     through a stride-0 access pattern (innermost step 1 keeps DVE 2x mode).
  -> y = f//10: shipped in-stream (replicated rows) with the bulk data.
Per-chunk single DMA stream (uint8, bitcast views):
  [ g0 bf16 | g1 bf16 | y fp16 | (n0,n1) fp8 pairs ]  (8*FC bytes/partition)
One-shot "smalls" DMA: 7 fp16 diag matrices + xsmall + fp8 DoubleRow weights.
Device dataflow per chunk:
    DVE: P1=g0*x P2=g1*y u=P1+P2 XU=x*u YU=y*u P3=g0*y P4=g1*x (bf16 2x)
    PE:  d1m  = V0*g0 + V1*g1 - V2*u            ( = -dot1 )
         NEG  = -(n0+n1) [DoubleRow fp8] + O0*g1 - O1*g0 + O2*P3 - O2*P4
                + O0*YU - O1*XU               ( = g.BW - n0 - n1 = -r2 )
    ACT: negb/d1b PSUM->bf16 copies; DVE rho = d1b*negb; ACT gelu(-rho)
    with accum_out -> [128, NCHUNK] partials, host sums in float64.
Sharding: pure data parallel, 2 batches per core on partition halves.
"""

import numpy as np
import ml_dtypes

import concourse.bacc as bacc
import concourse.bass as bass
import concourse.tile as tile
from concourse import mybir
from concourse.bass_utils import run_bass_kernel_spmd

B, H, W = 16, 480, 640
NPIX = H * W            # 307200
NCORES = 8
BPC = B // NCORES       # 2 batches per core
PHALF = 64              # partitions per batch
CPP = W // PHALF        # 10 image columns per partition
FTOT = H * CPP          # 4800 free elems per partition
CHUNKS = [480, 960, 960, 960, 960, 480]
NCHUNK = len(CHUNKS)
FCMAX = max(CHUNKS)
MMF = 512               # max matmul free dim (one PSUM bank)

F32 = mybir.dt.float32
F16 = mybir.dt.float16
BF16 = mybir.dt.bfloat16
F8 = mybir.dt.float8e4
U8 = mybir.dt.uint8
AF = mybir.ActivationFunctionType

# fp16 diag slots
D_O0, D_O1N, D_O2, D_O2N, D_V0, D_V1, D_V2N = range(7)
NDIAG = 7
# smalls byte offsets
SM_DG = 0                       # 7*128 fp16 = 1792 B
SM_X = SM_DG + NDIAG * 128 * 2  # 10 fp16 = 20 B
SM_WDR = SM_X + CPP * 2         # 256 fp8 = 256 B
SM_BYTES = SM_WDR + 256


def _build_kernel(tc, gn, smalls, out):
    nc = tc.nc
    gn_t = gn.ap()

    with (
        tc.tile_pool(name="singles", bufs=1) as singles,
        tc.tile_pool(name="ins", bufs=3) as ins,
        tc.tile_pool(name="mids", bufs=2) as mids,
        tc.tile_pool(name="psum", bufs=2, space="PSUM") as psp,
    ):
        sm = singles.tile([128, SM_BYTES], U8, name="sm")
        acc = singles.tile([128, NCHUNK], F32, name="acc")

        nc.scalar.dma_start(out=sm, in_=smalls.ap())
        dgv = sm[:, SM_DG : SM_DG + NDIAG * 256].bitcast(F16).rearrange(
            "p (d m) -> p d m", d=NDIAG
        )
        DG = [dgv[:, i, :] for i in range(NDIAG)]
        xsv = sm[:, SM_X : SM_X + 2 * CPP].bitcast(F16)          # [128, 10]
        wdrv = sm[:, SM_WDR : SM_WDR + 256].bitcast(F8).rearrange(
            "p (k m) -> p k m", k=2
        )

        f0s = [sum(CHUNKS[:i]) for i in range(NCHUNK)]
        for ci in range(NCHUNK):
            FC = CHUNKS[ci]
            f0 = f0s[ci]
            NC10 = FC // CPP
            gnt = ins.tile([128, 8 * FCMAX], U8, tag="gn", name=f"gn_{ci}")
            nc.sync.dma_start(
                out=gnt[:, : 8 * FC], in_=gn_t[:, 8 * f0 : 8 * f0 + 8 * FC]
            )
            g0 = gnt[:, 0 : 2 * FC].bitcast(BF16)
            g1 = gnt[:, 2 * FC : 4 * FC].bitcast(BF16)
            yc = gnt[:, 4 * FC : 6 * FC].bitcast(F16)
            npair = gnt[:, 6 * FC : 8 * FC].bitcast(F8).rearrange(
                "p (f k) -> p k f", k=2
            )
            # x through a stride-0 AP over the periodic [128,10] tile
            xc = xsv.unsqueeze(1).broadcast_to([128, NC10, CPP])
            g0_3 = g0.rearrange("p (c j) -> p c j", j=CPP)
            g1_3 = g1.rearrange("p (c j) -> p c j", j=CPP)

            def mtile(tag, dt=BF16):
                return mids.tile([128, FCMAX], dt, tag=tag, name=f"{tag}_{ci}")[:, :FC]

            def as3(ap):
                return ap.rearrange("p (c j) -> p c j", j=CPP)

            P1 = mtile("P1")
            nc.vector.tensor_mul(out=as3(P1), in0=g0_3, in1=xc)
            P2 = mtile("P2")
            nc.vector.tensor_mul(out=P2, in0=g1, in1=yc)
            u = mtile("u")
            nc.vector.tensor_add(out=u, in0=P1, in1=P2)
            P3 = mtile("P3")
            nc.vector.tensor_mul(out=P3, in0=g0, in1=yc)
            P4 = mtile("P4")
            nc.vector.tensor_mul(out=as3(P4), in0=g1_3, in1=xc)
            XU = mtile("XU")
            nc.vector.tensor_mul(out=as3(XU), in0=as3(u), in1=xc)
            YU = mtile("YU")
            nc.vector.tensor_mul(out=YU, in0=yc, in1=u)

            # PE: NEG = -(n0+n1) + g.BW ; d1m = V0*g0 + V1*g1 - V2*u
            neg_ps = psp.tile([128, FCMAX], F32, tag="neg", name=f"neg_{ci}")[:, :FC]
            d1_ps = psp.tile([128, FCMAX], F32, tag="d1", name=f"d1_{ci}")[:, :FC]

            # DoubleRow: out[m,f] = sum_k wdr[c,(k,m)]*npair[c,k,f] = -(n0+n1)
            for f0m in range(0, FC, MMF):
                fs = slice(f0m, min(f0m + MMF, FC))
                nc.tensor.matmul(
                    neg_ps[:, fs], wdrv, npair[:, :, fs],
                    start=True, stop=False,
                    perf_mode=mybir.MatmulPerfMode.DoubleRow,
                )
            neg_terms = [
                (D_O0, g1), (D_O1N, g0),
                (D_O2, P3), (D_O2N, P4),
                (D_O0, YU), (D_O1N, XU),
            ]
            for i, (di, rhs) in enumerate(neg_terms):
                for f0m in range(0, FC, MMF):
                    fs = slice(f0m, min(f0m + MMF, FC))
                    nc.tensor.matmul(
                        neg_ps[:, fs], DG[di], rhs[:, fs],
                        start=False, stop=(i == len(neg_terms) - 1),
                    )
            d1_terms = [(D_V0, g0), (D_V1, g1), (D_V2N, u)]
            for i, (di, rhs) in enumerate(d1_terms):
                for f0m in range(0, FC, MMF):
                    fs = slice(f0m, min(f0m + MMF, FC))
                    nc.tensor.matmul(
                        d1_ps[:, fs], DG[di], rhs[:, fs],
                        start=(i == 0), stop=(i == len(d1_terms) - 1),
                    )

            negb = mtile("negb")
            nc.scalar.activation(out=negb, in_=neg_ps, func=AF.Copy)
            d1b = mtile("d1b")
            nc.scalar.activation(out=d1b, in_=d1_ps, func=AF.Copy)

            rho = mtile("rho")
            nc.vector.tensor_mul(out=rho, in0=d1b, in1=negb)
            gl = mtile("gl")
            nc.scalar.activation(
                out=gl, in_=rho, func=AF.Gelu, bias=0.0, scale=-1.0,
                accum_out=acc[:, ci : ci + 1],
            )

        nc.sync.dma_start(out=out.ap(), in_=acc)


def build_bass():
    nc = bacc.Bacc("TRN2", target_bir_lowering=False, debug=False)
    gn = nc.dram_tensor("gn", [128, 8 * FTOT], U8, kind="ExternalInput")
    smalls = nc.dram_tensor("smalls", [128, SM_BYTES], U8, kind="ExternalInput")
    out = nc.dram_tensor("acc_out", [128, NCHUNK], F32, kind="ExternalOutput")
    with tile.TileContext(nc) as tc:
        _build_kernel(tc, gn, smalls, out)
    nc.compile()
    return nc


def make_in_maps(pose, grad_dirs, normal_flow):
    pose = np.asarray(pose, np.float32)
    gd = np.ascontiguousarray(np.asarray(grad_dirs, np.float32))
    nf = np.ascontiguousarray(np.asarray(normal_flow, np.float32))

    yrow = (np.arange(FTOT, dtype=np.int64) // CPP).astype(np.float16)
    f0s = [sum(CHUNKS[:i]) for i in range(NCHUNK)]

    def col_interleave(a, dtype):
        # [BPC, 2, H, W] -> [128, 2, FTOT]; partition = 64*h + x//10,
        # f = y*10 + x%10
        return np.ascontiguousarray(
            a.reshape(BPC, 2, H, PHALF, CPP)
            .transpose(0, 3, 1, 2, 4)
            .reshape(128, 2, FTOT)
            .astype(dtype)
        )

    in_maps = []
    for core in range(NCORES):
        b0 = core * BPC
        gdc = col_interleave(gd[b0 : b0 + BPC], ml_dtypes.bfloat16)
        nfc = col_interleave(nf[b0 : b0 + BPC], ml_dtypes.float8_e4m3)
        # packed per-chunk stream: [g0 | g1 | y | (n0,n1) pairs]
        gn = np.empty((128, 8 * FTOT), np.uint8)
        for ci in range(NCHUNK):
            FC = CHUNKS[ci]
            f0 = f0s[ci]
            b = 8 * f0
            sl = slice(f0, f0 + FC)
            gn[:, b : b + 2 * FC] = gdc[:, 0, sl].view(np.uint8)
            gn[:, b + 2 * FC : b + 4 * FC] = gdc[:, 1, sl].view(np.uint8)
            gn[:, b + 4 * FC : b + 6 * FC] = np.broadcast_to(
                yrow[sl].view(np.uint8), (128, 2 * FC)
            )
            gn[:, b + 6 * FC : b + 8 * FC] = np.ascontiguousarray(
                nfc[:, :, sl].transpose(0, 2, 1)
            ).reshape(128, 2 * FC).view(np.uint8)

        coef = np.zeros((NDIAG, 128), np.float32)
        for h in range(BPC):
            V = pose[b0 + h, :3]
            O = pose[b0 + h, 3:]
            rows = slice(h * PHALF, (h + 1) * PHALF)
            coef[D_O0, rows] = O[0]
            coef[D_O1N, rows] = -O[1]
            coef[D_O2, rows] = O[2]
            coef[D_O2N, rows] = -O[2]
            coef[D_V0, rows] = V[0]
            coef[D_V1, rows] = V[1]
            coef[D_V2N, rows] = -V[2]
        dgh = np.zeros((128, NDIAG, 128), np.float16)
        for c in range(128):
            dgh[c, :, c] = coef[:, c].astype(np.float16)
        xsmall = (
            (np.arange(128) % PHALF)[:, None] * CPP + np.arange(CPP)[None, :]
        ).astype(np.float16)
        wdrh = np.zeros((128, 2, 128), ml_dtypes.float8_e4m3)
        for c in range(128):
            wdrh[c, 0, c] = -1.0
            wdrh[c, 1, c] = -1.0
        smalls = np.empty((128, SM_BYTES), np.uint8)
        smalls[:, SM_DG : SM_DG + NDIAG * 256] = dgh.reshape(128, -1).view(np.uint8)
        smalls[:, SM_X : SM_X + 2 * CPP] = xsmall.view(np.uint8)
        smalls[:, SM_WDR : SM_WDR + 256] = wdrh.reshape(128, -1).view(np.uint8)
        in_maps.append({"gn": gn, "smalls": smalls})
    return in_maps


_NC_CACHE = None


def _get_nc():
    global _NC_CACHE
    if _NC_CACHE is None:
        _NC_CACHE = build_bass()
    return _NC_CACHE


def kernel(pose, grad_dirs, normal_flow):
    nc = _get_nc()
    in_maps = make_in_maps(pose, grad_dirs, normal_flow)
    res = run_bass_kernel_spmd(nc, in_maps, core_ids=list(range(NCORES)))
    total = 0.0
    for r in res.results:
        total += r["acc_out"].astype(np.float64).sum()
    return np.float32(total / (B * H * W))
